# revision 1
# baseline (speedup 1.0000x reference)
"""Causal self-attention (B=2, T=2048, C=2048, H=16) on 8 TRN2 NeuronCores.

Sharding: data-parallel over batch (2) x tensor-parallel over heads (4 heads
per core). Each core computes, for its batch element b and head group g:
  QKV projection for its heads' columns, causal attention for its 4 heads,
  and a partial output projection (row-sharded W_proj). The host sums the
  4 partial projections per batch element.

Device layouts (per core, fp16 compute / fp32 PSUM accumulation):
  xT   [C, T]      x_b transposed (host-side transpose)
  wqk  [C, 1024]   [Wq_h0..h3 | Wk_h0..h3], 128 cols per head
  wv   [C, 512]    Wv_h0..h3
  wp   [512, C]    W_proj rows for this head group
  out  [T, C] fp32 partial projection output

Attention per (head, 512-wide q-chunk), exploiting causality via loop
bounds and 4 precomputed diagonal masks:
  S^T[kt, q] = K_kt^T.T @ Q^T            (PE, one matmul per key tile kt)
  P^T = exp(scale * S^T)                 (ACT, PSUM->SBUF fp16)
  Y[q, d+1] += P^T.T @ [V | ones]        (PE, accumulated over kt in PSUM;
                                          the ones column yields the softmax
                                          denominator for free)
  y = Y[:, :d] * (1 / Y[:, d])           (DVE, per-partition scalar)
  y^T via PE transpose -> yt[d, head, t] (layout the projection consumes)
The N=129 AV matmuls trade some PE efficiency for a fully local pipeline
(no cross-engine reduction chains); measured, this keeps the PE ~98% busy
between phases and the HAM clock-gate warm.
"""

import os

import numpy as np

N_HEAD = 16
N_EMBD = 2048
B = 2
T = 2048
C = N_EMBD
D = C // N_HEAD  # 128
HPC = N_HEAD // 4  # heads per core = 4
N_CORES = 8
CT = C // 128  # 16 contraction tiles
TT = T // 128  # 16 t tiles
NCH = T // 512  # 4 chunks of 512

LAST_EXEC_NS = None

_CACHE = {}


def _build_nc():
    import concourse.bass as bass  # noqa: F401
    import concourse.tile as tile
    from concourse import bacc, mybir

    F32 = mybir.dt.float32
    F16 = mybir.dt.float16
    Exp = mybir.ActivationFunctionType.Exp
    Copy = mybir.ActivationFunctionType.Copy
    SCALE = 1.0 / float(np.sqrt(D))

    nc = bacc.Bacc("TRN2", target_bir_lowering=False, num_devices=N_CORES)

    xT_d = nc.dram_tensor("xT", [C, T], F16, kind="ExternalInput")
    wqk_d = nc.dram_tensor("wqk", [C, 8 * 128], F16, kind="ExternalInput")
    wv_d = nc.dram_tensor("wv", [C, 4 * 128], F16, kind="ExternalInput")
    wp_d = nc.dram_tensor("wp", [4 * 128, C], F16, kind="ExternalInput")
    out_d = nc.dram_tensor("out_part", [T, C], F16, kind="ExternalOutput")

    # Constants baked into the NEFF: diagonal causal masks and identity.
    kk = np.arange(128)[:, None]
    qq = np.arange(512)[None, :]
    masks = np.stack(
        [(qq >= (128 * i + kk)).astype(np.float16) for i in range(4)]
    )  # [4, 128, 512]
    masks_d = nc.inline_tensor(np.ascontiguousarray(masks), name="diagmasks")
    ident_d = nc.inline_tensor(np.eye(128, dtype=np.float16), name="ident128")

    with tile.TileContext(nc) as tc:
        with (
            tc.tile_pool(name="singles", bufs=1) as singles,
            tc.tile_pool(name="xtp", bufs=32) as xtp,
            tc.tile_pool(name="ptp", bufs=6) as ptp,
            tc.tile_pool(name="ysb", bufs=4) as ysbp,
            tc.tile_pool(name="rp", bufs=4) as rp,
            tc.tile_pool(name="ost", bufs=3) as ostp,
            tc.tile_pool(name="ps", bufs=4, space="PSUM") as ps,
            tc.tile_pool(name="yps", bufs=4, space="PSUM") as yps,
        ):
            # Per-c-tile weight loads, interleaved with the first x chunk, so
            # the first matmuls wait on ~512 KB, not the whole input set.
            wqk_t = []
            wv_t = []
            xt0 = []
            for c in range(CT):
                xc = xtp.tile([128, 512], F16, tag="xt", name=f"xt0_{c}")
                nc.sync.dma_start(out=xc, in_=xT_d[c * 128 : (c + 1) * 128, 0:512])
                xt0.append(xc)
                w = singles.tile([128, 8 * 128], F16, name=f"wqkc{c}")
                wqk_t.append(w)
            for ct in range(8):
                for c in range(CT):
                    nc.sync.dma_start(
                        out=wqk_t[c][:, ct * 128 : (ct + 1) * 128],
                        in_=wqk_d[c * 128 : (c + 1) * 128, ct * 128 : (ct + 1) * 128],
                    )

            # qkt: [d, coltile, t]; coltiles 0..3 = Q heads, 4..7 = K heads
            qkt_sb = singles.tile([128, 8, T], F16)
            # v with a ones column per (kt, head): [kt-tile, head, 129]
            vv_sb = singles.tile([128, TT, HPC, 129], F16)
            # y transposed: [d, head, t]
            yt_sb = singles.tile([128, HPC, T], F16)
            wp_sb = None
            mask_sb = None
            ident_sb = None

            # ---- Phase 1: QKV projection ----
            for tj in range(NCH):
                if tj == 0:
                    xt = xt0
                else:
                    xt = []
                    for c in range(CT):
                        xc = xtp.tile([128, 512], F16, tag="xt", name=f"xt{tj}_{c}")
                        nc.sync.dma_start(
                            out=xc,
                            in_=xT_d[
                                c * 128 : (c + 1) * 128, tj * 512 : (tj + 1) * 512
                            ],
                        )
                        xt.append(xc)
                for ct in range(8):
                    pq = ps.tile([128, 512], F32, tag="ps", name=f"pq{tj}_{ct}")
                    for c in range(CT):
                        nc.tensor.matmul(
                            pq,
                            wqk_t[c][:, ct * 128 : (ct + 1) * 128],
                            xt[c],
                            start=(c == 0),
                            stop=(c == CT - 1),
                        )
                    nc.scalar.activation(
                        out=qkt_sb[:, ct, tj * 512 : (tj + 1) * 512],
                        in_=pq,
                        func=Copy,
                    )
                if tj == 0:
                    # wv is first needed here; its DMA trails wqk/xt0.
                    for c in range(CT):
                        w = singles.tile([128, 512], F16, name=f"wvc{c}")
                        nc.sync.dma_start(out=w, in_=wv_d[c * 128 : (c + 1) * 128, :])
                        wv_t.append(w)
                for tt in range(4):
                    kt = tj * 4 + tt
                    pv = ps.tile([128, 512], F32, tag="ps", name=f"pv{kt}")
                    for c in range(CT):
                        nc.tensor.matmul(
                            pv,
                            xt[c][:, tt * 128 : (tt + 1) * 128],
                            wv_t[c],
                            start=(c == 0),
                            stop=(c == CT - 1),
                        )
                    nc.scalar.activation(
                        out=vv_sb[:, kt, :, 0:128],
                        in_=pv.rearrange("p (h d) -> p h d", h=HPC),
                        func=Copy,
                    )
                    nc.vector.memset(vv_sb[:, kt, :, 128:129], 1.0)
                if tj == 0:
                    # First needed by attention; loaded during phase 1.
                    wp_sb = singles.tile([128, HPC, C], F16, name="wp_sb")
                    nc.sync.dma_start(
                        out=wp_sb,
                        in_=wp_d[:, :].rearrange("(a p) n -> p a n", p=128),
                    )
                    mask_sb = singles.tile([128, 4, 512], F16, name="mask_sb")
                    nc.sync.dma_start(
                        out=mask_sb, in_=masks_d[:, :, :].rearrange("a p n -> p a n")
                    )
                    ident_sb = singles.tile([128, 128], F16, name="ident_sb")
                    nc.sync.dma_start(out=ident_sb, in_=ident_d[:, :])

            # ---- Phases 2+3 interleaved per q-chunk ----
            for j in range(NCH):
                for h in range(HPC):
                    y_tiles = [
                        yps.tile([128, 129], F32, tag="y", name=f"ytile{h}_{j}_{qs}")
                        for qs in range(4)
                    ]
                    for kt in range(4 * j + 4):
                        di = kt - 4 * j
                        lo = 128 * di if di > 0 else 0
                        ss = ps.tile([128, 512], F32, tag="ps", name=f"ss{h}{j}{kt}")
                        nc.tensor.matmul(
                            ss[:, lo:],
                            qkt_sb[:, 4 + h, kt * 128 : (kt + 1) * 128],
                            qkt_sb[:, h, j * 512 + lo : (j + 1) * 512],
                            start=True,
                            stop=True,
                        )
                        pt = ptp.tile([128, 512], F16, tag="pt", name=f"pt{h}{j}{kt}")
                        nc.scalar.activation(
                            out=pt[:, lo:], in_=ss[:, lo:], func=Exp, scale=SCALE
                        )
                        if di >= 0:
                            nc.vector.tensor_mul(
                                pt[:, lo : lo + 128],
                                pt[:, lo : lo + 128],
                                mask_sb[:, di, lo : lo + 128],
                            )
                        for qs in range(max(0, di), 4):
                            nc.tensor.matmul(
                                y_tiles[qs],
                                pt[:, qs * 128 : (qs + 1) * 128],
                                vv_sb[:, kt, h, :],
                                start=(kt == 0),
                                stop=(kt == 4 * j + qs),
                            )
                    for qs in range(4):
                        yt = y_tiles[qs]
                        r = rp.tile([128, 1], F32, tag="r", name=f"r{h}{j}{qs}")
                        nc.vector.reciprocal(r, yt[:, 128:129])
                        y16 = ysbp.tile([128, 128], F16, tag="y16", name=f"y16_{qs}")
                        nc.vector.tensor_scalar_mul(y16, yt[:, 0:128], r)
                        ytp = yps.tile([128, 128], F16, tag="y", name=f"ytp{h}{j}{qs}")
                        nc.tensor.transpose(ytp, y16, ident_sb)
                        tglob = (j * 4 + qs) * 128
                        nc.scalar.activation(
                            out=yt_sb[:, h, tglob : tglob + 128], in_=ytp, func=Copy
                        )

                for tt in range(4 * j, 4 * j + 4):
                    ot = ostp.tile([128, C], F16, tag="ot", name=f"ot{tt}")
                    for cc in range(4):
                        po = ps.tile([128, 512], F32, tag="ps", name=f"po{tt}_{cc}")
                        for hd in range(HPC):
                            nc.tensor.matmul(
                                po,
                                yt_sb[:, hd, tt * 128 : (tt + 1) * 128],
                                wp_sb[:, hd, cc * 512 : (cc + 1) * 512],
                                start=(hd == 0),
                                stop=(hd == HPC - 1),
                            )
                        if cc % 2 == 0:
                            nc.vector.tensor_copy(
                                out=ot[:, cc * 512 : (cc + 1) * 512], in_=po
                            )
                        else:
                            nc.scalar.activation(
                                out=ot[:, cc * 512 : (cc + 1) * 512], in_=po, func=Copy
                            )
                            nc.sync.dma_start(
                                out=out_d[
                                    tt * 128 : (tt + 1) * 128,
                                    (cc - 1) * 512 : (cc + 1) * 512,
                                ],
                                in_=ot[:, (cc - 1) * 512 : (cc + 1) * 512],
                            )

    nc.compile()
    return nc


def _get_nc():
    if "nc" not in _CACHE:
        _CACHE["nc"] = _build_nc()
    return _CACHE["nc"]


def kernel(x, W_attn, W_proj):
    global LAST_EXEC_NS
    from concourse.bass_utils import run_bass_kernel_spmd

    x = np.asarray(x)
    W_attn = np.asarray(W_attn)
    W_proj = np.asarray(W_proj)

    in_maps = []
    for core in range(N_CORES):
        b, g = divmod(core, 4)
        heads = range(4 * g, 4 * g + 4)
        xT = np.ascontiguousarray(x[b].T).astype(np.float16)
        wqk = np.concatenate(
            [W_attn[:, h * D : (h + 1) * D] for h in heads]
            + [W_attn[:, C + h * D : C + (h + 1) * D] for h in heads],
            axis=1,
        ).astype(np.float16)
        wv = np.concatenate(
            [W_attn[:, 2 * C + h * D : 2 * C + (h + 1) * D] for h in heads], axis=1
        ).astype(np.float16)
        wp = W_proj[4 * g * D : 4 * (g + 1) * D, :].astype(np.float16)
        in_maps.append({"xT": xT, "wqk": wqk, "wv": wv, "wp": wp})

    nc = _get_nc()
    res = run_bass_kernel_spmd(
        nc,
        in_maps,
        list(range(N_CORES)),
        trace=bool(os.environ.get("KERNEL_TRACE")),
    )
    LAST_EXEC_NS = res.exec_time_ns

    out = np.zeros((B, T, C), dtype=np.float32)
    for core in range(N_CORES):
        b = core // 4
        out[b] += res.results[core]["out_part"].astype(np.float32)
    return out



# revision 2
# speedup vs baseline: 1.1618x; 1.1618x over previous
"""Causal self-attention (B=2, T=2048, C=2048, H=16) on 8 TRN2 NeuronCores.

Sharding: data-parallel over batch (2) x tensor-parallel over heads (4 heads
per core). Each core computes, for its batch element b and head group g:
  QKV projection for its heads' columns, causal attention for its 4 heads,
  and a partial output projection (row-sharded W_proj). The host sums the
  4 partial projections per batch element.

Key performance structure (v2):
  - All inputs are host-prepacked into partition-major layouts so every DMA
    moves >=4KB contiguous per partition line (the v1 kernel's 256B-line
    weight loads ran at ~50 GB/s and starved the PE for the first ~100us).
    DMAs are issued in exact first-use order on the sync ring.
  - A short warm-up spin of dummy matmuls keeps the PE HAM clock-gate at
    K=8/8 (2.4 GHz) while the first input DMAs land.
  - QKV projection, attention, and output projection are interleaved per
    512-row chunk j: [K-head groups, V groups, (Q_h group, attention_h)x4,
    proj]. Attention q-chunk j only needs K/V tiles kt <= 4j+3, all of which
    exist after chunk j's projection. The interleave keeps the PE streaming
    while the ACT engine chews the exp() backlog.
  - The y^T transpose needed by the output projection is done by the DMA
    xbar (dma_start_transpose), not the PE.

Per-core device layouts (fp16 compute / fp32 PSUM accumulation):
  xb   [128, 4, 16, 512]  x^T tiles, chunk-major: [p, tj, c, t]
  wqk  [128, 8, 16, 128]  [p, coltile, c, n]; coltiles 0..3 Q heads, 4..7 K
  wv   [128, 16, 512]     [p, c, (h d)]
  wp   [128, 4, 2048]     [p, h, c]  W_proj rows for this head group
  out  [T, C] fp16 partial projection output

Attention per (head, 512-wide q-chunk): S^T = K_kt^T.T @ Q^T per key tile,
P^T = exp(scale*S^T) (ACT), diagonal masks on DVE, Y[q, d+1] += P^T.T @
[V | ones] accumulated in PSUM (the ones column gives the softmax
denominator), y = Y[:, :d] * (1/Y[:, d]) on DVE, then DMA-transpose into
yt[d, h, t] for the projection.
"""

import os

import numpy as np

N_HEAD = 16
N_EMBD = 2048
B = 2
T = 2048
C = N_EMBD
D = C // N_HEAD  # 128
HPC = N_HEAD // 4  # heads per core = 4
N_CORES = 8
CT = C // 128  # 16 contraction tiles
TT = T // 128  # 16 t tiles
NCH = T // 512  # 4 chunks of 512
NWARM = 32

LAST_EXEC_NS = None

_CACHE = {}


def _build_nc():
    import concourse.bass as bass  # noqa: F401
    import concourse.tile as tile
    from concourse import bacc, mybir

    F32 = mybir.dt.float32
    F16 = mybir.dt.float16
    Exp = mybir.ActivationFunctionType.Exp
    Copy = mybir.ActivationFunctionType.Copy
    SCALE = 1.0 / float(np.sqrt(D))

    nc = bacc.Bacc("TRN2", target_bir_lowering=False, num_devices=N_CORES)

    xb_d = nc.dram_tensor("xb", [128, NCH, CT, 512], F16, kind="ExternalInput")
    wqk_d = nc.dram_tensor("wqk", [128, 8, CT, 128], F16, kind="ExternalInput")
    wv_d = nc.dram_tensor("wv", [128, CT, 512], F16, kind="ExternalInput")
    wp_d = nc.dram_tensor("wp", [128, HPC, C], F16, kind="ExternalInput")
    out_d = nc.dram_tensor("out_part", [T, C], F16, kind="ExternalOutput")

    # Diagonal causal masks, partition-major: [128 k, diag idx, 512 q].
    kk = np.arange(128)[:, None]
    qq = np.arange(512)[None, :]
    masks = np.stack(
        [(qq >= (128 * i + kk)).astype(np.float16) for i in range(4)], axis=1
    )  # [128, 4, 512]
    masks_d = nc.inline_tensor(np.ascontiguousarray(masks), name="diagmasks")

    with tile.TileContext(nc) as tc:
        with (
            tc.tile_pool(name="singles", bufs=1) as singles,
            tc.tile_pool(name="xbp", bufs=3) as xbp,
            tc.tile_pool(name="ptp", bufs=6) as ptp,
            tc.tile_pool(name="ysb", bufs=4) as ysbp,
            tc.tile_pool(name="rp", bufs=4) as rp,
            tc.tile_pool(name="ost", bufs=3) as ostp,
            tc.tile_pool(name="ps", bufs=4, space="PSUM") as ps,
            tc.tile_pool(name="yps", bufs=4, space="PSUM") as yps,
        ):
            # ---- PE warm-up: keep the HAM clock-gate at 8/8 while DMAs land
            warm_w = singles.tile([128, 128], F16, name="warm_w")
            warm_x = singles.tile([128, 512], F16, name="warm_x")
            nc.vector.memset(warm_w, 0.0)
            nc.vector.memset(warm_x, 0.0)
            for i in range(NWARM):
                wps = ps.tile([128, 512], F32, tag="ps", name=f"warm{i}")
                nc.tensor.matmul(wps, warm_w, warm_x, start=True, stop=True)

            # ---- Input DMAs, issued in first-use order on the sync ring ----
            wqk_sb = singles.tile([128, 8, CT, 128], F16, name="wqk_sb")
            wv_sb = singles.tile([128, CT, 512], F16, name="wv_sb")
            wp_sb = singles.tile([128, HPC, C], F16, name="wp_sb")
            mask_sb = singles.tile([128, 4, 512], F16, name="mask_sb")
            xb_t = [None] * NCH

            nc.sync.dma_start(out=wqk_sb[:, 4], in_=wqk_d[:, 4])  # first K group
            xb_t[0] = xbp.tile([128, CT, 512], F16, tag="xb", name="xb0")
            nc.sync.dma_start(out=xb_t[0], in_=xb_d[:, 0])
            for ct in (5, 6, 7):
                nc.sync.dma_start(out=wqk_sb[:, ct], in_=wqk_d[:, ct])
            nc.sync.dma_start(out=wv_sb, in_=wv_d[:, :])
            for ct in (0, 1, 2, 3):
                nc.sync.dma_start(out=wqk_sb[:, ct], in_=wqk_d[:, ct])
            nc.sync.dma_start(out=mask_sb, in_=masks_d[:, :, :])
            nc.sync.dma_start(out=wp_sb, in_=wp_d[:, :])
            xb_t[1] = xbp.tile([128, CT, 512], F16, tag="xb", name="xb1")
            nc.sync.dma_start(out=xb_t[1], in_=xb_d[:, 1])

            # qkt: [d, coltile, t]; coltiles 0..3 = Q heads, 4..7 = K heads
            qkt_sb = singles.tile([128, 8, T], F16)
            # v with a ones column per (kt, head): [kt-tile, head, 129]
            vv_sb = singles.tile([128, TT, HPC, 129], F16)
            # y transposed: [d, head, t]
            yt_sb = singles.tile([128, HPC, T], F16)

            def qkv_group(tj, ct):
                # projection group for coltile ct of chunk tj (N=512, 16 MMs)
                xt = xb_t[tj]
                pq = ps.tile([128, 512], F32, tag="ps", name=f"pq{tj}_{ct}")
                for c in range(CT):
                    nc.tensor.matmul(
                        pq,
                        wqk_sb[:, ct, c, :],
                        xt[:, c, :],
                        start=(c == 0),
                        stop=(c == CT - 1),
                    )
                nc.vector.tensor_copy(
                    out=qkt_sb[:, ct, tj * 512 : (tj + 1) * 512], in_=pq
                )

            def v_group(tj, tt):
                kt = tj * 4 + tt
                xt = xb_t[tj]
                pv = ps.tile([128, 512], F32, tag="ps", name=f"pv{kt}")
                for c in range(CT):
                    nc.tensor.matmul(
                        pv,
                        xt[:, c, tt * 128 : (tt + 1) * 128],
                        wv_sb[:, c, :],
                        start=(c == 0),
                        stop=(c == CT - 1),
                    )
                nc.vector.tensor_copy(
                    out=vv_sb[:, kt, :, 0:128],
                    in_=pv.rearrange("p (h d) -> p h d", h=HPC),
                )
                nc.vector.memset(vv_sb[:, kt, :, 128:129], 1.0)

            def attn_head(j, h):
                # causal attention for head h over q-chunk j
                y_tiles = [
                    yps.tile([128, 129], F32, tag="y", name=f"ytile{h}_{j}_{qs}")
                    for qs in range(4)
                ]
                for kt in range(4 * j + 4):
                    di = kt - 4 * j
                    lo = 128 * di if di > 0 else 0
                    ss = ps.tile([128, 512], F32, tag="ps", name=f"ss{h}{j}{kt}")
                    nc.tensor.matmul(
                        ss[:, lo:],
                        qkt_sb[:, 4 + h, kt * 128 : (kt + 1) * 128],
                        qkt_sb[:, h, j * 512 + lo : (j + 1) * 512],
                        start=True,
                        stop=True,
                    )
                    pt = ptp.tile([128, 512], F16, tag="pt", name=f"pt{h}{j}{kt}")
                    nc.scalar.activation(
                        out=pt[:, lo:], in_=ss[:, lo:], func=Exp, scale=SCALE
                    )
                    if di >= 0:
                        nc.vector.tensor_mul(
                            pt[:, lo : lo + 128],
                            pt[:, lo : lo + 128],
                            mask_sb[:, di, lo : lo + 128],
                        )
                    for qs in range(max(0, di), 4):
                        nc.tensor.matmul(
                            y_tiles[qs],
                            pt[:, qs * 128 : (qs + 1) * 128],
                            vv_sb[:, kt, h, :],
                            start=(kt == 0),
                            stop=(kt == 4 * j + qs),
                        )
                for qs in range(4):
                    yt = y_tiles[qs]
                    r = rp.tile([128, 1], F32, tag="r", name=f"r{h}{j}{qs}")
                    nc.vector.reciprocal(r, yt[:, 128:129])
                    y16 = ysbp.tile([128, 128], F16, tag="y16", name=f"y16_{qs}")
                    nc.vector.tensor_scalar_mul(y16, yt[:, 0:128], r)
                    tglob = (j * 4 + qs) * 128
                    nc.sync.dma_start_transpose(
                        out=yt_sb[:, h, tglob : tglob + 128], in_=y16
                    )

            def proj_tile(tt):
                ot = ostp.tile([128, C], F16, tag="ot", name=f"ot{tt}")
                for cc in range(4):
                    po = ps.tile([128, 512], F32, tag="ps", name=f"po{tt}_{cc}")
                    for hd in range(HPC):
                        nc.tensor.matmul(
                            po,
                            yt_sb[:, hd, tt * 128 : (tt + 1) * 128],
                            wp_sb[:, hd, cc * 512 : (cc + 1) * 512],
                            start=(hd == 0),
                            stop=(hd == HPC - 1),
                        )
                    if cc % 2 == 0:
                        nc.vector.tensor_copy(
                            out=ot[:, cc * 512 : (cc + 1) * 512], in_=po
                        )
                    else:
                        nc.scalar.activation(
                            out=ot[:, cc * 512 : (cc + 1) * 512], in_=po, func=Copy
                        )
                        nc.sync.dma_start(
                            out=out_d[
                                tt * 128 : (tt + 1) * 128,
                                (cc - 1) * 512 : (cc + 1) * 512,
                            ],
                            in_=ot[:, (cc - 1) * 512 : (cc + 1) * 512],
                        )

            for tj in range(NCH):
                for ct in (4, 5, 6, 7):  # K heads first: attention needs them
                    qkv_group(tj, ct)
                for tt in range(4):
                    v_group(tj, tt)
                for h in range(HPC):
                    qkv_group(tj, h)  # Q head h
                    attn_head(tj, h)
                # prefetch x chunk tj+2 (buffer of chunk tj-1 is free now)
                nxt = tj + 2
                if nxt < NCH:
                    xb_t[nxt] = xbp.tile(
                        [128, CT, 512], F16, tag="xb", name=f"xb{nxt}"
                    )
                    nc.sync.dma_start(out=xb_t[nxt], in_=xb_d[:, nxt])
                for tt in range(4 * tj, 4 * tj + 4):
                    proj_tile(tt)

    nc.compile()
    return nc


def _get_nc():
    if "nc" not in _CACHE:
        _CACHE["nc"] = _build_nc()
    return _CACHE["nc"]


def kernel(x, W_attn, W_proj):
    global LAST_EXEC_NS
    from concourse.bass_utils import run_bass_kernel_spmd

    x = np.asarray(x)
    W_attn = np.asarray(W_attn)
    W_proj = np.asarray(W_proj)

    in_maps = []
    for core in range(N_CORES):
        b, g = divmod(core, 4)
        heads = range(4 * g, 4 * g + 4)
        # x^T tiles, chunk-major: [p, tj, c, t]
        xb = (
            np.ascontiguousarray(x[b].T)
            .astype(np.float16)
            .reshape(CT, 128, NCH, 512)
            .transpose(1, 2, 0, 3)
        )
        wqk = np.concatenate(
            [W_attn[:, h * D : (h + 1) * D] for h in heads]
            + [W_attn[:, C + h * D : C + (h + 1) * D] for h in heads],
            axis=1,
        ).astype(np.float16)  # [C, 1024]
        wqk = wqk.reshape(CT, 128, 8, 128).transpose(1, 2, 0, 3)  # [p, ct, c, n]
        wv = np.concatenate(
            [W_attn[:, 2 * C + h * D : 2 * C + (h + 1) * D] for h in heads], axis=1
        ).astype(np.float16)  # [C, 512]
        wv = wv.reshape(CT, 128, 512).transpose(1, 0, 2)  # [p, c, n]
        wp = W_proj[4 * g * D : 4 * (g + 1) * D, :].astype(np.float16)  # [512, C]
        wp = wp.reshape(HPC, 128, C).transpose(1, 0, 2)  # [p, h, c]
        in_maps.append(
            {
                "xb": np.ascontiguousarray(xb),
                "wqk": np.ascontiguousarray(wqk),
                "wv": np.ascontiguousarray(wv),
                "wp": np.ascontiguousarray(wp),
            }
        )

    nc = _get_nc()
    res = run_bass_kernel_spmd(
        nc,
        in_maps,
        list(range(N_CORES)),
        trace=bool(os.environ.get("KERNEL_TRACE")),
    )
    LAST_EXEC_NS = res.exec_time_ns

    out = np.zeros((B, T, C), dtype=np.float32)
    for core in range(N_CORES):
        b = core // 4
        out[b] += res.results[core]["out_part"].astype(np.float32)
    return out


# revision 4
# speedup vs baseline: 1.2703x; 1.0934x over previous
"""Causal self-attention (B=2, T=2048, C=2048, H=16) on 8 TRN2 NeuronCores.

Sharding: data-parallel over batch (2) x tensor-parallel over heads (4 heads
per core). Each core computes, for its batch element b and head group g:
  QKV projection for its heads' columns, causal attention for its 4 heads,
  and a partial output projection (row-sharded W_proj). The host sums the
  4 partial projections per batch element.

Key performance structure (v3):
  - All inputs host-prepacked into partition-major layouts so every DMA
    moves >=4KB contiguous per partition line, issued in first-use order.
  - A warm-up spin of dummy matmuls holds the PE HAM clock-gate at 8/8
    (2.4 GHz) while the first input DMAs land.
  - Software-pipelined chunk schedule: chunk j's QKV projection groups are
    interleaved at ~4us granularity with attention SEGMENTS (4 key-tiles)
    of chunk j-1 and the output projection of chunk j-2. Every cross-engine
    dependency (PSUM->DVE qkt copy, ACT exp backlog, DMA transpose) is a
    full chunk old by the time the PE consumes it, so the PE never waits.
  - The y^T transpose needed by the output projection runs on the DMA xbar
    (dma_start_transpose), not the PE.

Per-core device layouts (fp16 compute / fp32 PSUM accumulation):
  xb   [128, 4, 16, 512]  x^T tiles, chunk-major: [p, tj, c, t]
  wqk  [128, 8, 16, 128]  [p, coltile, c, n]; coltiles 0..3 Q heads, 4..7 K
  wv   [128, 16, 512]     [p, c, (h d)]
  wp   [128, 4, 2048]     [p, h, c]  W_proj rows for this head group
  out  [T, C] fp16 partial projection output

Attention per (head, 512-wide q-chunk): S^T = K_kt^T.T @ Q^T per key tile,
P^T = exp(scale*S^T) (ACT), diagonal masks on DVE, Y[q, d+1] += P^T.T @
[V | ones] accumulated in PSUM (the ones column gives the softmax
denominator), y = Y[:, :d] * (1/Y[:, d]) on DVE, then DMA-transpose into
yt[d, h, t] for the projection.
"""

import os

import numpy as np

N_HEAD = 16
N_EMBD = 2048
B = 2
T = 2048
C = N_EMBD
D = C // N_HEAD  # 128
HPC = N_HEAD // 4  # heads per core = 4
N_CORES = 8
CT = C // 128  # 16 contraction tiles
TT = T // 128  # 16 t tiles
NCH = T // 512  # 4 chunks of 512
NWARM = 32

LAST_EXEC_NS = None

_CACHE = {}


def _build_nc():
    import concourse.bass as bass  # noqa: F401
    import concourse.tile as tile
    from concourse import bacc, mybir

    F32 = mybir.dt.float32
    F16 = mybir.dt.float16
    Exp = mybir.ActivationFunctionType.Exp
    Copy = mybir.ActivationFunctionType.Copy
    SCALE = 1.0 / float(np.sqrt(D))

    nc = bacc.Bacc("TRN2", target_bir_lowering=False, num_devices=N_CORES)

    xb_d = nc.dram_tensor("xb", [128, NCH, CT, 512], F16, kind="ExternalInput")
    wqk_d = nc.dram_tensor("wqk", [128, 8, CT, 128], F16, kind="ExternalInput")
    wv_d = nc.dram_tensor("wv", [128, CT, 512], F16, kind="ExternalInput")
    wp_d = nc.dram_tensor("wp", [128, HPC, C], F16, kind="ExternalInput")
    out_d = nc.dram_tensor("out_part", [T, C], F16, kind="ExternalOutput")

    # Diagonal causal masks, partition-major: [128 k, diag idx, 512 q].
    kk = np.arange(128)[:, None]
    qq = np.arange(512)[None, :]
    masks = np.stack(
        [(qq >= (128 * i + kk)).astype(np.float16) for i in range(4)], axis=1
    )  # [128, 4, 512]
    masks_d = nc.inline_tensor(np.ascontiguousarray(masks), name="diagmasks")

    with tile.TileContext(nc) as tc:
        with (
            tc.tile_pool(name="singles", bufs=1) as singles,
            tc.tile_pool(name="xbp", bufs=3) as xbp,
            tc.tile_pool(name="ptp", bufs=6) as ptp,
            tc.tile_pool(name="ysb", bufs=4) as ysbp,
            tc.tile_pool(name="rp", bufs=4) as rp,
            tc.tile_pool(name="ost", bufs=3) as ostp,
            tc.tile_pool(name="ps", bufs=4, space="PSUM") as ps,
            tc.tile_pool(name="yps", bufs=4, space="PSUM") as yps,
        ):
            # ---- PE warm-up: keep the HAM clock-gate at 8/8 while DMAs land
            warm_w = singles.tile([128, 128], F16, name="warm_w")
            warm_x = singles.tile([128, 512], F16, name="warm_x")
            nc.vector.memset(warm_w, 0.0)
            nc.vector.memset(warm_x, 0.0)
            for i in range(NWARM):
                wps = ps.tile([128, 512], F32, tag="ps", name=f"warm{i}")
                nc.tensor.matmul(wps, warm_w, warm_x, start=True, stop=True)

            # ---- Input DMAs, issued in first-use order on the sync ring ----
            wqk_sb = singles.tile([128, 8, CT, 128], F16, name="wqk_sb")
            wv_sb = singles.tile([128, CT, 512], F16, name="wv_sb")
            wp_sb = singles.tile([128, HPC, C], F16, name="wp_sb")
            mask_sb = singles.tile([128, 4, 512], F16, name="mask_sb")
            xb_t = [None] * NCH

            nc.sync.dma_start(out=wqk_sb[:, 4], in_=wqk_d[:, 4])  # first K group
            xb_t[0] = xbp.tile([128, CT, 512], F16, tag="xb", name="xb0")
            nc.sync.dma_start(out=xb_t[0], in_=xb_d[:, 0])
            for ct in (5, 6, 7):
                nc.sync.dma_start(out=wqk_sb[:, ct], in_=wqk_d[:, ct])
            nc.sync.dma_start(out=wv_sb, in_=wv_d[:, :])
            for ct in (0, 1, 2, 3):
                nc.sync.dma_start(out=wqk_sb[:, ct], in_=wqk_d[:, ct])
            nc.sync.dma_start(out=mask_sb, in_=masks_d[:, :, :])
            nc.sync.dma_start(out=wp_sb, in_=wp_d[:, :])
            xb_t[1] = xbp.tile([128, CT, 512], F16, tag="xb", name="xb1")
            nc.sync.dma_start(out=xb_t[1], in_=xb_d[:, 1])

            # qkt: [d, coltile, t]; coltiles 0..3 = Q heads, 4..7 = K heads
            qkt_sb = singles.tile([128, 8, T], F16)
            # v with a ones column per (kt, head): [kt-tile, head, 129]
            vv_sb = singles.tile([128, TT, HPC, 129], F16)
            # y transposed: [d, head, t]
            yt_sb = singles.tile([128, HPC, T], F16)

            def qkv_group(tj, ct):
                # projection group for coltile ct of chunk tj (N=512, 16 MMs)
                xt = xb_t[tj]
                pq = ps.tile([128, 512], F32, tag="ps", name=f"pq{tj}_{ct}")
                for c in range(CT):
                    nc.tensor.matmul(
                        pq,
                        wqk_sb[:, ct, c, :],
                        xt[:, c, :],
                        start=(c == 0),
                        stop=(c == CT - 1),
                    )
                nc.vector.tensor_copy(
                    out=qkt_sb[:, ct, tj * 512 : (tj + 1) * 512], in_=pq
                )

            def v_group(tj, tt):
                kt = tj * 4 + tt
                xt = xb_t[tj]
                pv = ps.tile([128, 512], F32, tag="ps", name=f"pv{kt}")
                for c in range(CT):
                    nc.tensor.matmul(
                        pv,
                        xt[:, c, tt * 128 : (tt + 1) * 128],
                        wv_sb[:, c, :],
                        start=(c == 0),
                        stop=(c == CT - 1),
                    )
                nc.vector.tensor_copy(
                    out=vv_sb[:, kt, :, 0:128],
                    in_=pv.rearrange("p (h d) -> p h d", h=HPC),
                )
                nc.vector.memset(vv_sb[:, kt, :, 128:129], 1.0)

            y_live = {}  # h -> y_tiles for the attention chunk in flight

            def attn_seg(j, h, sg):
                # key-tile segment sg (4 kt) of head h, q-chunk j
                if sg == 0:
                    y_live[h] = [
                        yps.tile([128, 129], F32, tag="y", name=f"yt{h}_{j}_{qs}")
                        for qs in range(4)
                    ]
                y_tiles = y_live[h]
                for kt in range(4 * sg, 4 * sg + 4):
                    di = kt - 4 * j
                    lo = 128 * di if di > 0 else 0
                    ss = ps.tile([128, 512], F32, tag="ps", name=f"ss{h}{j}{kt}")
                    nc.tensor.matmul(
                        ss[:, lo:],
                        qkt_sb[:, 4 + h, kt * 128 : (kt + 1) * 128],
                        qkt_sb[:, h, j * 512 + lo : (j + 1) * 512],
                        start=True,
                        stop=True,
                    )
                    pt = ptp.tile([128, 512], F16, tag="pt", name=f"pt{h}{j}{kt}")
                    nc.scalar.activation(
                        out=pt[:, lo:], in_=ss[:, lo:], func=Exp, scale=SCALE
                    )
                    if di >= 0:
                        nc.vector.tensor_mul(
                            pt[:, lo : lo + 128],
                            pt[:, lo : lo + 128],
                            mask_sb[:, di, lo : lo + 128],
                        )
                    for qs in range(max(0, di), 4):
                        nc.tensor.matmul(
                            y_tiles[qs],
                            pt[:, qs * 128 : (qs + 1) * 128],
                            vv_sb[:, kt, h, :],
                            start=(kt == 0),
                            stop=(kt == 4 * j + qs),
                        )

            def attn_head_end(j, h):
                y_tiles = y_live.pop(h)
                for qs in range(4):
                    yt = y_tiles[qs]
                    r = rp.tile([128, 1], F32, tag="r", name=f"r{h}{j}{qs}")
                    nc.vector.reciprocal(r, yt[:, 128:129])
                    y16 = ysbp.tile([128, 128], F16, tag="y16", name=f"y16_{qs}")
                    nc.vector.tensor_scalar_mul(y16, yt[:, 0:128], r)
                    tglob = (j * 4 + qs) * 128
                    nc.sync.dma_start_transpose(
                        out=yt_sb[:, h, tglob : tglob + 128], in_=y16
                    )

            def proj_tile(tt, fine_store=False):
                ot = ostp.tile([128, C], F16, tag="ot", name=f"ot{tt}")
                for cc in range(4):
                    po = ps.tile([128, 512], F32, tag="ps", name=f"po{tt}_{cc}")
                    for hd in range(HPC):
                        nc.tensor.matmul(
                            po,
                            yt_sb[:, hd, tt * 128 : (tt + 1) * 128],
                            wp_sb[:, hd, cc * 512 : (cc + 1) * 512],
                            start=(hd == 0),
                            stop=(hd == HPC - 1),
                        )
                    if cc % 2 == 0:
                        nc.vector.tensor_copy(
                            out=ot[:, cc * 512 : (cc + 1) * 512], in_=po
                        )
                    else:
                        nc.scalar.activation(
                            out=ot[:, cc * 512 : (cc + 1) * 512], in_=po, func=Copy
                        )
                    if fine_store:
                        nc.sync.dma_start(
                            out=out_d[
                                tt * 128 : (tt + 1) * 128,
                                cc * 512 : (cc + 1) * 512,
                            ],
                            in_=ot[:, cc * 512 : (cc + 1) * 512],
                        )
                    elif cc % 2 == 1:
                        nc.sync.dma_start(
                            out=out_d[
                                tt * 128 : (tt + 1) * 128,
                                (cc - 1) * 512 : (cc + 1) * 512,
                            ],
                            in_=ot[:, (cc - 1) * 512 : (cc + 1) * 512],
                        )

            def chunk_groups(tj):
                for ct in (4, 5, 6, 7):
                    yield ("qkv", ct)
                for tt in range(4):
                    yield ("v", tt)
                for h in range(HPC):
                    yield ("qkv", h)

            # ---- chunk 0: pure QKV ----
            for kind, a in chunk_groups(0):
                (qkv_group if kind == "qkv" else v_group)(0, a)

            # ---- chunks 1..3: QKV(j) x attn(j-1) segments x proj(j-2) ----
            for j in range(1, NCH):
                if j + 1 < NCH:
                    xb_t[j + 1] = xbp.tile(
                        [128, CT, 512], F16, tag="xb", name=f"xb{j + 1}"
                    )
                    nc.sync.dma_start(out=xb_t[j + 1], in_=xb_d[:, j + 1])
                aj = j - 1
                segs = [(h, s) for h in range(HPC) for s in range(aj + 1)]
                si = 0
                for gi, (kind, a) in enumerate(chunk_groups(j)):
                    (qkv_group if kind == "qkv" else v_group)(j, a)
                    want = (gi + 1) * len(segs) // 12
                    while si < want:
                        h, s = segs[si]
                        attn_seg(aj, h, s)
                        if s == aj:
                            attn_head_end(aj, h)
                        si += 1
                if j >= 2:
                    for tt in range(4 * (j - 2), 4 * (j - 2) + 4):
                        proj_tile(tt)

            # ---- epilogue: attn(3) segments x proj(2), then proj(3) ----
            aj = NCH - 1
            nonfinal = 0
            for h in range(HPC):
                for s in range(aj + 1):
                    attn_seg(aj, h, s)
                    if s == aj:
                        attn_head_end(aj, h)
                    else:
                        nonfinal += 1
                        if nonfinal % 3 == 0:
                            proj_tile(4 * (NCH - 2) + nonfinal // 3 - 1)
            for tt in range(4 * (NCH - 1), 4 * NCH):
                proj_tile(tt, fine_store=(tt == 4 * NCH - 1))

    nc.compile()
    return nc


def _get_nc():
    if "nc" not in _CACHE:
        _CACHE["nc"] = _build_nc()
    return _CACHE["nc"]


def kernel(x, W_attn, W_proj):
    global LAST_EXEC_NS
    from concourse.bass_utils import run_bass_kernel_spmd

    x = np.asarray(x)
    W_attn = np.asarray(W_attn)
    W_proj = np.asarray(W_proj)

    in_maps = []
    for core in range(N_CORES):
        b, g = divmod(core, 4)
        heads = range(4 * g, 4 * g + 4)
        # x^T tiles, chunk-major: [p, tj, c, t]
        xb = (
            np.ascontiguousarray(x[b].T)
            .astype(np.float16)
            .reshape(CT, 128, NCH, 512)
            .transpose(1, 2, 0, 3)
        )
        wqk = np.concatenate(
            [W_attn[:, h * D : (h + 1) * D] for h in heads]
            + [W_attn[:, C + h * D : C + (h + 1) * D] for h in heads],
            axis=1,
        ).astype(np.float16)  # [C, 1024]
        wqk = wqk.reshape(CT, 128, 8, 128).transpose(1, 2, 0, 3)  # [p, ct, c, n]
        wv = np.concatenate(
            [W_attn[:, 2 * C + h * D : 2 * C + (h + 1) * D] for h in heads], axis=1
        ).astype(np.float16)  # [C, 512]
        wv = wv.reshape(CT, 128, 512).transpose(1, 0, 2)  # [p, c, n]
        wp = W_proj[4 * g * D : 4 * (g + 1) * D, :].astype(np.float16)  # [512, C]
        wp = wp.reshape(HPC, 128, C).transpose(1, 0, 2)  # [p, h, c]
        in_maps.append(
            {
                "xb": np.ascontiguousarray(xb),
                "wqk": np.ascontiguousarray(wqk),
                "wv": np.ascontiguousarray(wv),
                "wp": np.ascontiguousarray(wp),
            }
        )

    nc = _get_nc()
    res = run_bass_kernel_spmd(
        nc,
        in_maps,
        list(range(N_CORES)),
        trace=bool(os.environ.get("KERNEL_TRACE")),
    )
    LAST_EXEC_NS = res.exec_time_ns

    out = np.zeros((B, T, C), dtype=np.float32)
    for core in range(N_CORES):
        b = core // 4
        out[b] += res.results[core]["out_part"].astype(np.float32)
    return out
